# revision 19
# baseline (speedup 1.0000x reference)
"""Trainium2 Bass kernel for EpipolarAttention (B=2, C=3, H=W=64, N=4096).

Factorization (validated against the reference to ~5e-6 absmax-rel):
  d_epipolar[i,j]^2 = ||a_i||^2 - (a_i . u_j)^2        (cross-product identity)
                    = alpha_i . beta_j                  (rank-6 bilinear form)
  with alpha = [ax^2,ay^2,az^2,2axay,2axaz,2ayaz],
       beta  = [1-ux^2,1-uy^2,1-uz^2,-uxuy,-uxuz,-uyuz],
       a_i = f_src[:,i]-o_proj, u_j = diff_j/||diff_j||.

  Row softmax (axis j): E1 = exp(50*d - 50*||a_i||)     (bound shift, no max pass)
  W = 1 - E1/rowsum(E1); Aw = A*W with A = f_src^T f_tar (rank-3 matmul)
  Col softmax (axis i): P = exp(Aw - 20) (static shift), colsum all-reduced over
  the 8 cores (each owns 512 rows of i), normalization folded into the final
  matmul rhs: attended^T[c,i] = sum_j (f_src[c,j]/colsum[j]) * P^T[j,i].

Sharding: each of the 8 cores processes a 512-row i-stripe of both batches.
Everything stays in SBUF; the only cross-core traffic is a 16KB AllReduce of
colsum per batch.
"""
import os
import numpy as np

B, C, H, Wd = 2, 3, 64, 64
N = H * Wd                     # 4096
NCORES = 8
ST = N // NCORES               # 512 rows per core
NIB = ST // 128                # 4 partition blocks per stripe
JT = N // 128                  # 32 transpose column chunks
NQ = 4                         # quarters of the 4096-wide row (1024 each)
SH = 50.0                      # sharpness
MSHIFT = 20.0                  # static shift for the column softmax
EPS_D2 = 1e-3                  # clamps fp-negative d^2 before sqrt

# layout of the packed constant input "cst" [128, CW] (f32 words per partition)
OFF_EYE = B * N + B * ST       # 9216: identity [128,128]
OFF_GSRC = OFF_EYE + 128       # 9344: gsrc [128, B*96]
OFF_BIAS = OFF_GSRC + B * JT * 3   # 9536: bias1 [128, B*4]
OFF_EPS = OFF_BIAS + B * NIB   # 9544: [eps, -MSHIFT]
CW = OFF_EPS + 2               # 9546

_cache = {}


def _build(reps=1):
    import concourse.bass as bass
    import concourse.bacc as bacc
    import concourse.mybir as mybir
    import concourse.tile as tile

    f32 = mybir.dt.float32
    AF = mybir.ActivationFunctionType
    OP = mybir.AluOpType

    # Bacc (not raw Bass): its compile() splits multi-waits into event
    # semaphores — HW instructions carry at most one sync wait.
    nc = bacc.Bacc("TRN2", target_bir_lowering=False, num_devices=NCORES)

    # All constants ship as ONE dram tensor loaded by ONE DMA, so every
    # first consumer carries a single sync wait (walrus allows only one on
    # quadrant-tiled matmuls).
    d_cst = nc.dram_tensor("cst", [128, CW], f32, kind="ExternalInput")
    d_out = nc.dram_tensor("outp", [B, 3, ST], f32, kind="ExternalOutput")

    with tile.TileContext(nc) as tc:
        with (
            tc.tile_pool(name="consts", bufs=1) as consts,
            tc.tile_pool(name="rowA", bufs=4) as rowA,      # d then Aw
            tc.tile_pool(name="rowB", bufs=4) as rowB,      # E1/W then P^T groups
            tc.tile_pool(name="small", bufs=2) as small,
            tc.tile_pool(name="psmm", bufs=2, space="PSUM") as psmm,
            tc.tile_pool(name="pstr", bufs=2, space="PSUM") as pstr,
            tc.tile_pool(name="psatt", bufs=2, space="PSUM") as psatt,
            tc.tile_pool(name="dram", bufs=4, space="DRAM") as drampool,
        ):
            # ---- load constants (single DMA) ------------------------------------
            # Few-partition matmul operands share partition offsets so they
            # don't each burn their free-range on all 128 partitions. PE
            # requires lhsT/rhs to share a base partition in {0,32,64,96}:
            # alpha+beta at base 0, fsrc+ftar at base 32.
            bigc = consts.tile([128, CW], f32, tag="bigc")
            nc.sync.dma_start(out=bigc[:], in_=d_cst.ap())
            beta_sb = bigc[0:6, 0:B * N].rearrange("c (b n) -> c b n", b=B)
            alpha_sb = bigc[0:6, B * N:B * N + B * ST].rearrange("c (b n) -> c b n", b=B)
            ftar_sb = bigc[32:35, 0:B * N].rearrange("c (b n) -> c b n", b=B)
            fsrc_sb = bigc[32:35, B * N:B * N + B * ST].rearrange("c (b n) -> c b n", b=B)
            eye_sb = bigc[:, OFF_EYE:OFF_EYE + 128]
            gsrc_sb = bigc[:, OFF_GSRC:OFF_GSRC + B * JT * 3].rearrange(
                "p (b i) -> p b i", b=B)
            bias1_sb = bigc[:, OFF_BIAS:OFF_BIAS + B * NIB].rearrange(
                "p (b i) -> p b i", b=B)
            epsb_sb = bigc[:, OFF_EPS:OFF_EPS + 1]
            mshift_sb = bigc[:, OFF_EPS + 1:OFF_EPS + 2]

            for _rep in range(reps):
              for b in range(B):
                # ---- phase B: d = sqrt(alpha.beta + eps), E1 = exp(50d - 50|a|),
                #      rs = rowsum(E1) via accum_out --------------------------------
                d_tiles = []
                for ib in range(NIB):
                    d_t = rowA.tile([128, N], f32, tag="rowA")
                    for q in range(NQ):
                        ps = psmm.tile([128, 1024], f32, tag="mm")
                        for h in range(2):
                            jb = q * 2 + h
                            nc.tensor.matmul(
                                ps[:, h * 512:(h + 1) * 512],
                                lhsT=alpha_sb[:, b, ib * 128:(ib + 1) * 128],
                                rhs=beta_sb[:, b, jb * 512:(jb + 1) * 512],
                                start=True, stop=True,
                            )
                        nc.scalar.activation(
                            d_t[:, q * 1024:(q + 1) * 1024], ps[:],
                            AF.Sqrt, bias=epsb_sb[:], scale=1.0,
                        )
                    d_tiles.append(d_t)

                rs_t = small.tile([128, NIB], f32, tag="rs")
                e1_tiles = []
                for ib in range(NIB):
                    e1_t = rowB.tile([128, N], f32, tag="rowB")
                    nc.scalar.activation(
                        e1_t[:], d_tiles[ib][:],
                        AF.Exp, bias=bias1_sb[:, b, ib:ib + 1], scale=SH,
                        accum_out=rs_t[:, ib:ib + 1],
                    )
                    e1_tiles.append(e1_t)

                negir_t = small.tile([128, NIB], f32, tag="negir")
                nc.vector.reciprocal(negir_t[:], rs_t[:])
                nc.vector.tensor_scalar_mul(negir_t[:], negir_t[:], -1.0)

                # ---- phase C: W = 1 - E1/rs ; Aw = A * W -------------------------
                aw_tiles = []
                for ib in range(NIB):
                    # W = 1 - E1/rs, in place over E1
                    w_t = e1_tiles[ib]
                    nc.vector.tensor_scalar(
                        w_t[:], w_t[:],
                        scalar1=negir_t[:, ib:ib + 1], scalar2=1.0,
                        op0=OP.mult, op1=OP.add,
                    )
                    aw_t = rowA.tile([128, N], f32, tag="rowA")
                    for q in range(NQ):
                        ps = psmm.tile([128, 1024], f32, tag="mm")
                        for h in range(2):
                            jb = q * 2 + h
                            nc.tensor.matmul(
                                ps[:, h * 512:(h + 1) * 512],
                                lhsT=fsrc_sb[:, b, ib * 128:(ib + 1) * 128],
                                rhs=ftar_sb[:, b, jb * 512:(jb + 1) * 512],
                                start=True, stop=True,
                            )
                        nc.vector.tensor_tensor(
                            aw_t[:, q * 1024:(q + 1) * 1024], ps[:],
                            w_t[:, q * 1024:(q + 1) * 1024], op=OP.mult,
                        )
                    aw_tiles.append(aw_t)

                # ---- phase T: P^T = exp(Aw^T - 20) via PE transpose + fused exp;
                #      colsum partials via accum_out -------------------------------
                colsum_t = small.tile([128, JT], f32, tag="colsum")
                pt_tiles = [
                    rowB.tile([128, N], f32, tag="rowB", name=f"pt_b{b}_{g}")
                    for g in range(NIB)
                ]
                for jt in range(JT):
                    pst = pstr.tile([128, 512], f32, tag="tr")
                    for ib in range(NIB):
                        nc.tensor.transpose(
                            pst[:, ib * 128:(ib + 1) * 128],
                            aw_tiles[ib][:, jt * 128:(jt + 1) * 128],
                            eye_sb[:],
                        )
                    g_, k_ = divmod(jt, 8)
                    nc.scalar.activation(
                        pt_tiles[g_][:, k_ * 512:(k_ + 1) * 512], pst[:],
                        AF.Exp, bias=mshift_sb[:], scale=1.0,
                        accum_out=colsum_t[:, jt:jt + 1],
                    )

                # ---- phase R: AllReduce colsum over the 8 cores ------------------
                cc_in = drampool.tile([128, JT], f32, tag="ccin")
                cc_out = drampool.tile([128, JT], f32, tag="ccout")
                nc.sync.dma_start(out=cc_in[:], in_=colsum_t[:])
                nc.gpsimd.collective_compute(
                    "AllReduce", OP.add,
                    replica_groups=[list(range(NCORES))],
                    ins=[cc_in.opt()], outs=[cc_out.opt()],
                )
                cstot_t = small.tile([128, JT], f32, tag="cstot")
                nc.sync.dma_start(out=cstot_t[:], in_=cc_out[:])

                icol_t = small.tile([128, JT], f32, tag="icol")
                nc.vector.reciprocal(icol_t[:], cstot_t[:])
                # g[p, (jt,c)] = f_srcT[p, (jt,c)] / colsum[p, jt]
                g_t = small.tile([128, JT, 3], f32, tag="gt")
                icol_rep = bass.AP(
                    tensor=icol_t[:].tensor,
                    offset=icol_t[:].offset,
                    ap=[icol_t[:].ap[0], icol_t[:].ap[1], [0, 3]],
                )
                nc.vector.tensor_tensor(
                    g_t[:],
                    gsrc_sb[:, b].rearrange("p (t c) -> p t c", c=3),
                    icol_rep, op=OP.mult,
                )

                # ---- phase F: attended^T[c,i] = sum_j g[j,c] P^T[j,i] ------------
                ps_att = psatt.tile([3, ST], f32, tag="att")
                for jt in range(JT):
                    g_, k_ = divmod(jt, 8)
                    nc.tensor.matmul(
                        ps_att[:],
                        lhsT=g_t[:, jt, :],
                        rhs=pt_tiles[g_][:, k_ * 512:(k_ + 1) * 512],
                        start=(jt == 0), stop=(jt == JT - 1),
                    )
                out_sb = small.tile([3, ST], f32, tag="outsb")
                nc.vector.tensor_copy(out_sb[:], ps_att[:])
                nc.sync.dma_start(out=d_out.ap()[b], in_=out_sb[:])

    nc.compile()
    return nc


def _host_prep(f_tar, f_src, K, R, t):
    """Geometry on host (CPU jax mirrors the reference's f32 ops), then the
    per-core device input arrays."""
    import jax
    import jax.numpy as jnp

    cpu = jax.devices("cpu")[0]

    def geom(K, R, t):
        ix, iy = jnp.meshgrid(jnp.arange(H), jnp.arange(Wd), indexing="ij")
        pix = jnp.stack(
            [ix.reshape(-1), iy.reshape(-1), jnp.ones(N, dtype=jnp.int32)], axis=0
        ).astype(jnp.float32)
        K_inv = jnp.linalg.inv(K)
        cam_rays = jnp.einsum("bij,jn->bin", K_inv, pix)
        tar_proj = jnp.einsum(
            "bij,bjn->bin", K, jnp.einsum("bij,bjn->bin", R, cam_rays) + t
        )
        o_proj = jnp.einsum("bij,bjn->bin", K, t)
        return tar_proj - o_proj, o_proj

    diff, o_proj = jax.jit(geom, backend="cpu")(
        jax.device_put(np.asarray(K), cpu),
        jax.device_put(np.asarray(R), cpu),
        jax.device_put(np.asarray(t), cpu),
    )
    diff = np.asarray(diff)
    o_proj = np.asarray(o_proj)

    f_src_flat = np.asarray(f_src).reshape(B, C, N)
    f_tar_flat = np.asarray(f_tar).reshape(B, C, N)

    diff_norm = np.sqrt((diff ** 2).sum(axis=1, keepdims=True)).astype(np.float32)
    u = (diff / diff_norm).astype(np.float32)
    a = (f_src_flat - o_proj).astype(np.float32)
    na = (a ** 2).sum(axis=1).astype(np.float32)          # (B,N)
    sqna = np.sqrt(na).astype(np.float32)

    alpha = np.stack(
        [a[:, 0] ** 2, a[:, 1] ** 2, a[:, 2] ** 2,
         2 * a[:, 0] * a[:, 1], 2 * a[:, 0] * a[:, 2], 2 * a[:, 1] * a[:, 2]],
        axis=1,
    ).astype(np.float32)                                   # (B,6,N)
    beta = np.stack(
        [1 - u[:, 0] ** 2, 1 - u[:, 1] ** 2, 1 - u[:, 2] ** 2,
         -u[:, 0] * u[:, 1], -u[:, 0] * u[:, 2], -u[:, 1] * u[:, 2]],
        axis=1,
    ).astype(np.float32)                                   # (B,6,N)

    bias1 = (-SH * sqna).astype(np.float32)                # (B,N)
    # gsrc[b, p, jt*3+c] = f_src[b, c, jt*128+p]
    gsrc = np.ascontiguousarray(
        f_src_flat.transpose(0, 2, 1).reshape(B, JT, 128, 3).transpose(0, 2, 1, 3)
    ).reshape(B, 128, JT * 3).astype(np.float32)
    eye = np.eye(128, dtype=np.float32)

    in_maps = []
    for r in range(NCORES):
        sl = slice(r * ST, (r + 1) * ST)
        cst = np.zeros((128, CW), dtype=np.float32)
        cst[0:6, 0:B * N] = beta.transpose(1, 0, 2).reshape(6, B * N)
        cst[0:6, B * N:B * N + B * ST] = (
            alpha[:, :, sl].transpose(1, 0, 2).reshape(6, B * ST))
        cst[32:35, 0:B * N] = f_tar_flat.transpose(1, 0, 2).reshape(3, B * N)
        cst[32:35, B * N:B * N + B * ST] = (
            f_src_flat[:, :, sl].transpose(1, 0, 2).reshape(3, B * ST))
        cst[:, OFF_EYE:OFF_EYE + 128] = eye
        cst[:, OFF_GSRC:OFF_GSRC + B * JT * 3] = (
            gsrc.transpose(1, 0, 2).reshape(128, B * JT * 3))
        cst[:, OFF_BIAS:OFF_BIAS + B * NIB] = (
            bias1[:, sl].reshape(B, NIB, 128).transpose(2, 0, 1).reshape(128, B * NIB))
        cst[:, OFF_EPS] = EPS_D2
        cst[:, OFF_EPS + 1] = -MSHIFT
        in_maps.append({"cst": cst})
    return in_maps


def _run(in_maps, trace=False, reps=1):
    from concourse.bass_utils import run_bass_kernel_spmd

    key = ("nc", reps)
    if key not in _cache:
        _cache[key] = _build(reps)
    res = run_bass_kernel_spmd(
        _cache[key], in_maps, list(range(NCORES)), trace=trace
    )
    return res


def kernel(f_tar, f_src, K, R, t, _reps=1):
    in_maps = _host_prep(f_tar, f_src, K, R, t)
    res = _run(in_maps, reps=_reps)
    att_T = np.empty((B, 3, N), dtype=np.float32)
    for r in range(NCORES):
        att_T[:, :, r * ST:(r + 1) * ST] = res.results[r]["outp"]
    out = att_T.transpose(0, 2, 1).reshape(B, C, H, Wd)
    kernel._last_results = res
    return out


# revision 26
# speedup vs baseline: 1.3931x; 1.3931x over previous
"""Trainium2 Bass kernel for EpipolarAttention (B=2, C=3, H=W=64, N=4096).

Factorization (validated against the reference to ~5e-6 absmax-rel):
  d_epipolar[i,j]^2 = ||a_i||^2 - (a_i . u_j)^2        (cross-product identity)
                    = alpha_i . beta_j                  (rank-6 bilinear form)
  with alpha = [ax^2,ay^2,az^2,2axay,2axaz,2ayaz],
       beta  = [1-ux^2,1-uy^2,1-uz^2,-uxuy,-uxuz,-uyuz],
       a_i = f_src[:,i]-o_proj, u_j = diff_j/||diff_j||.

  Row softmax (axis j): E1 = exp(50*d - 50*||a_i||)     (bound shift, no max pass)
  W = 1 - E1/rowsum(E1); Aw = A*W with A = f_src^T f_tar (rank-3 matmul)
  Col softmax (axis i): P = exp(Aw - 20) (static shift), colsum all-reduced over
  the 8 cores (each owns 512 rows of i), normalization folded into the final
  matmul rhs: attended^T[c,i] = sum_j (f_src[c,j]/colsum[j]) * P^T[j,i].

Sharding: each core processes a 512-row i-stripe of both batches. The final
contraction runs over j which must sit on PE partitions, so W (values in
[0,1]) is transposed via the 2-byte XBAR DMA transpose in bf16 and Aw^T is
rebuilt as A^T (rank-3 matmul, exact) * W^T. The only cross-core traffic is a
16KB AllReduce of colsum per batch.
"""
import numpy as np

B, C, H, Wd = 2, 3, 64, 64
N = H * Wd                     # 4096
NCORES = 8
ST = N // NCORES               # 512 rows per core
NIB = ST // 128                # 4 partition blocks per stripe
JT = N // 128                  # 32 column chunks of 128 (j on partitions)
NG = 4                         # P^T groups (8 column chunks each)
KG = JT // NG                  # 8 column chunks per group
SH = 50.0                      # sharpness
MSHIFT = 20.0                  # static shift for the column softmax
EPS_D2 = 1e-3                  # clamps fp-negative d^2 before sqrt

# per-batch constant layout "cst" [B, 128, CWB] (f32 words per partition)
OFF_GSRC = N + ST              # 4608: gsrc [128, 96]
OFF_BIAS = OFF_GSRC + JT * 3   # 4704: bias1 [128, 4]
OFF_EPS = OFF_BIAS + NIB       # 4708: [eps, -MSHIFT]
CWB = OFF_EPS + 2              # 4710

_cache = {}


def _build(reps=1, skip_collective=False):
    import concourse.bass as bass
    import concourse.bacc as bacc
    import concourse.mybir as mybir
    import concourse.tile as tile

    f32 = mybir.dt.float32
    bf16 = mybir.dt.bfloat16
    AF = mybir.ActivationFunctionType
    OP = mybir.AluOpType

    # Bacc (not raw Bass): its compile() splits multi-waits into event
    # semaphores — HW instructions carry at most one sync wait.
    nc = bacc.Bacc("TRN2", target_bir_lowering=False, num_devices=NCORES)

    d_cst = nc.dram_tensor("cst", [B, 128, CWB], f32, kind="ExternalInput")
    d_out = nc.dram_tensor("outp", [B, 3, ST], f32, kind="ExternalOutput")

    with tile.TileContext(nc) as tc:
        with (
            tc.tile_pool(name="consts", bufs=2) as consts,
            tc.tile_pool(name="rowd", bufs=2) as rowd,      # d -> E1 -> W (in place)
            tc.tile_pool(name="half", bufs=2) as half,      # W cast to bf16
            tc.tile_pool(name="halfT", bufs=4) as halfT,    # W^T groups (bf16)
            tc.tile_pool(name="rowP", bufs=4) as rowP,      # Aw^T -> P^T groups
            tc.tile_pool(name="small", bufs=2) as small,
            tc.tile_pool(name="psmm", bufs=3, space="PSUM") as psmm,
            tc.tile_pool(name="dram", bufs=2, space="DRAM") as drampool,
        ):
            for _rep in range(reps):
              for b in range(B):
                # ---- per-batch constants (single DMA). PE needs lhsT/rhs on
                # the same base partition in {0,32,64,96}: alpha+beta at base
                # 0, fsrc+ftar at base 32, at distinct free offsets.
                cstb = consts.tile([128, CWB], f32, tag="cstb")
                nc.sync.dma_start(out=cstb[:], in_=d_cst.ap()[b])
                beta_sb = cstb[0:6, 0:N]
                alpha_sb = cstb[0:6, N:N + ST]
                ftar_sb = cstb[32:35, 0:N]
                fsrc_sb = cstb[32:35, N:N + ST]
                gsrc_sb = cstb[:, OFF_GSRC:OFF_GSRC + JT * 3]
                bias1_sb = cstb[:, OFF_BIAS:OFF_BIAS + NIB]
                epsb_sb = cstb[:, OFF_EPS:OFF_EPS + 1]
                mshift_sb = cstb[:, OFF_EPS + 1:OFF_EPS + 2]

                # ---- phase B+C1 per i-block: d = sqrt(alpha.beta + eps);
                # E1 = exp(50d - 50|a|) in place (rs fused via accum_out);
                # W = 1 - E1/rs in place; cast bf16; XBAR-transpose ----------
                rs_t = small.tile([128, NIB], f32, tag="rs")
                nrs_t = small.tile([128, NIB], f32, tag="nrs")
                negir_t = small.tile([128, NIB], f32, tag="negir")
                hiT = [
                    halfT.tile([128, KG, NIB, 128], bf16, tag="hiT",
                               name=f"hiT_{b}_{g}")
                    for g in range(NG)
                ]
                for ib in range(NIB):
                    dE = rowd.tile([128, N], f32, tag="dE")
                    for q in range(4):
                        ps = psmm.tile([128, 1024], f32, tag="mm")
                        for h in range(2):
                            jb = q * 2 + h
                            nc.tensor.matmul(
                                ps[:, h * 512:(h + 1) * 512],
                                lhsT=alpha_sb[:, ib * 128:(ib + 1) * 128],
                                rhs=beta_sb[:, jb * 512:(jb + 1) * 512],
                                start=True, stop=True,
                            )
                        nc.scalar.activation(
                            dE[:, q * 1024:(q + 1) * 1024], ps[:],
                            AF.Sqrt, bias=epsb_sb, scale=1.0,
                        )
                    nc.scalar.activation(
                        dE[:], dE[:],
                        AF.Exp, bias=bias1_sb[:, ib:ib + 1], scale=SH,
                        accum_out=rs_t[:, ib:ib + 1],
                    )
                    nc.vector.tensor_scalar_mul(
                        nrs_t[:, ib:ib + 1], rs_t[:, ib:ib + 1], -1.0)
                    nc.vector.reciprocal(
                        negir_t[:, ib:ib + 1], nrs_t[:, ib:ib + 1])
                    nc.vector.tensor_scalar(
                        dE[:], dE[:],
                        scalar1=negir_t[:, ib:ib + 1], scalar2=1.0,
                        op0=OP.mult, op1=OP.add,
                    )
                    hi = half.tile([128, N], bf16, tag="half")
                    nc.gpsimd.tensor_copy(hi[:], dE[:])
                    for g in range(NG):
                        nc.sync.dma_start_transpose(
                            hiT[g][:, :, ib, :],
                            hi[:, g * 1024:(g + 1) * 1024],
                        )

                # ---- phase C2 per group: Aw^T = A^T * W^T; P^T = exp(Aw^T-20)
                # in place; colsum partials by free-axis reduce ---------------
                colsum_t = small.tile([128, JT], f32, tag="colsum")
                pt_tiles = []
                for g in range(NG):
                    awt = rowP.tile([128, N], f32, tag="rowP", name=f"awt_{b}_{g}")
                    for q in range(4):
                        ps = psmm.tile([128, 1024], f32, tag="mm")
                        for h in range(2):
                            jt = g * KG + q * 2 + h
                            nc.tensor.matmul(
                                ps[:, h * 512:(h + 1) * 512],
                                lhsT=ftar_sb[:, jt * 128:(jt + 1) * 128],
                                rhs=fsrc_sb[:],
                                start=True, stop=True,
                            )
                        nc.vector.tensor_tensor(
                            awt[:, q * 1024:(q + 1) * 1024], ps[:],
                            hiT[g][:, q * 2:q * 2 + 2, :, :].rearrange(
                                "p k i2 q2 -> p (k i2 q2)"),
                            op=OP.mult,
                        )
                    nc.scalar.activation(
                        awt[:], awt[:], AF.Exp, bias=mshift_sb, scale=1.0,
                    )
                    nc.vector.tensor_reduce(
                        colsum_t[:, g * KG:(g + 1) * KG],
                        awt[:].rearrange("p (k i) -> p k i", k=KG),
                        axis=mybir.AxisListType.X, op=OP.add,
                    )
                    pt_tiles.append(awt)

                # ---- phase R: AllReduce colsum over the 8 cores -------------
                cc_in = drampool.tile([128, JT], f32, tag="ccin")
                cc_out = drampool.tile([128, JT], f32, tag="ccout")
                nc.sync.dma_start(out=cc_in[:], in_=colsum_t[:])
                if skip_collective:
                    nc.sync.dma_start(out=cc_out[:], in_=cc_in[:])
                else:
                    nc.gpsimd.collective_compute(
                        "AllReduce", OP.add,
                        replica_groups=[list(range(NCORES))],
                        ins=[cc_in.opt()], outs=[cc_out.opt()],
                    )
                cstot_t = small.tile([128, JT], f32, tag="cstot")
                nc.sync.dma_start(out=cstot_t[:], in_=cc_out[:])

                icol_t = small.tile([128, JT], f32, tag="icol")
                nc.vector.reciprocal(icol_t[:], cstot_t[:])
                # g[p, (jt,c)] = f_srcT[p, (jt,c)] / colsum[p, jt]
                g_t = small.tile([128, JT, 3], f32, tag="gt")
                icol_rep = bass.AP(
                    tensor=icol_t[:].tensor,
                    offset=icol_t[:].offset,
                    ap=[icol_t[:].ap[0], icol_t[:].ap[1], [0, 3]],
                )
                nc.vector.tensor_tensor(
                    g_t[:],
                    gsrc_sb.rearrange("p (t c) -> p t c", c=3),
                    icol_rep, op=OP.mult,
                )

                # ---- phase F: attended^T[c,i] = sum_j g[j,c] P^T[j,i] -------
                ps_att = psmm.tile([128, 1024], f32, tag="mm", name=f"att_{b}")
                for jt in range(JT):
                    g_, k_ = divmod(jt, KG)
                    nc.tensor.matmul(
                        ps_att[0:3, 0:ST],
                        lhsT=g_t[:, jt, :],
                        rhs=pt_tiles[g_][:, k_ * 512:(k_ + 1) * 512],
                        start=(jt == 0), stop=(jt == JT - 1),
                    )
                out_sb = small.tile([3, ST], f32, tag="outsb")
                nc.vector.tensor_copy(out_sb[:], ps_att[0:3, 0:ST])
                nc.sync.dma_start(out=d_out.ap()[b], in_=out_sb[:])

    nc.compile()
    return nc


def _host_prep(f_tar, f_src, K, R, t):
    """Geometry on host (CPU jax mirrors the reference's f32 ops), then the
    per-core device input arrays."""
    import jax
    import jax.numpy as jnp

    cpu = jax.devices("cpu")[0]

    def geom(K, R, t):
        ix, iy = jnp.meshgrid(jnp.arange(H), jnp.arange(Wd), indexing="ij")
        pix = jnp.stack(
            [ix.reshape(-1), iy.reshape(-1), jnp.ones(N, dtype=jnp.int32)], axis=0
        ).astype(jnp.float32)
        K_inv = jnp.linalg.inv(K)
        cam_rays = jnp.einsum("bij,jn->bin", K_inv, pix)
        tar_proj = jnp.einsum(
            "bij,bjn->bin", K, jnp.einsum("bij,bjn->bin", R, cam_rays) + t
        )
        o_proj = jnp.einsum("bij,bjn->bin", K, t)
        return tar_proj - o_proj, o_proj

    diff, o_proj = jax.jit(geom, backend="cpu")(
        jax.device_put(np.asarray(K), cpu),
        jax.device_put(np.asarray(R), cpu),
        jax.device_put(np.asarray(t), cpu),
    )
    diff = np.asarray(diff)
    o_proj = np.asarray(o_proj)

    f_src_flat = np.asarray(f_src).reshape(B, C, N)
    f_tar_flat = np.asarray(f_tar).reshape(B, C, N)

    diff_norm = np.sqrt((diff ** 2).sum(axis=1, keepdims=True)).astype(np.float32)
    u = (diff / diff_norm).astype(np.float32)
    a = (f_src_flat - o_proj).astype(np.float32)
    na = (a ** 2).sum(axis=1).astype(np.float32)          # (B,N)
    sqna = np.sqrt(na).astype(np.float32)

    alpha = np.stack(
        [a[:, 0] ** 2, a[:, 1] ** 2, a[:, 2] ** 2,
         2 * a[:, 0] * a[:, 1], 2 * a[:, 0] * a[:, 2], 2 * a[:, 1] * a[:, 2]],
        axis=1,
    ).astype(np.float32)                                   # (B,6,N)
    beta = np.stack(
        [1 - u[:, 0] ** 2, 1 - u[:, 1] ** 2, 1 - u[:, 2] ** 2,
         -u[:, 0] * u[:, 1], -u[:, 0] * u[:, 2], -u[:, 1] * u[:, 2]],
        axis=1,
    ).astype(np.float32)                                   # (B,6,N)

    bias1 = (-SH * sqna).astype(np.float32)                # (B,N)
    # gsrc[b, p, jt*3+c] = f_src[b, c, jt*128+p]
    gsrc = np.ascontiguousarray(
        f_src_flat.transpose(0, 2, 1).reshape(B, JT, 128, 3).transpose(0, 2, 1, 3)
    ).reshape(B, 128, JT * 3).astype(np.float32)

    in_maps = []
    for r in range(NCORES):
        sl = slice(r * ST, (r + 1) * ST)
        cst = np.zeros((B, 128, CWB), dtype=np.float32)
        cst[:, 0:6, 0:N] = beta.astype(np.float32)
        cst[:, 0:6, N:N + ST] = alpha[:, :, sl]
        cst[:, 32:35, 0:N] = f_tar_flat
        cst[:, 32:35, N:N + ST] = f_src_flat[:, :, sl]
        cst[:, :, OFF_GSRC:OFF_GSRC + JT * 3] = gsrc
        cst[:, :, OFF_BIAS:OFF_BIAS + NIB] = (
            bias1[:, sl].reshape(B, NIB, 128).transpose(0, 2, 1))
        cst[:, :, OFF_EPS] = EPS_D2
        cst[:, :, OFF_EPS + 1] = -MSHIFT
        in_maps.append({"cst": cst})
    return in_maps


def _run(in_maps, reps=1, skip_collective=False):
    from concourse.bass_utils import run_bass_kernel_spmd

    key = ("nc", reps, skip_collective)
    if key not in _cache:
        _cache[key] = _build(reps, skip_collective)
    res = run_bass_kernel_spmd(
        _cache[key], in_maps, list(range(NCORES))
    )
    return res


def kernel(f_tar, f_src, K, R, t, _reps=1):
    in_maps = _host_prep(f_tar, f_src, K, R, t)
    res = _run(in_maps, reps=_reps)
    att_T = np.empty((B, 3, N), dtype=np.float32)
    for r in range(NCORES):
        att_T[:, :, r * ST:(r + 1) * ST] = res.results[r]["outp"]
    out = att_T.transpose(0, 2, 1).reshape(B, C, H, Wd)
    kernel._last_results = res
    return out


# revision 27
# speedup vs baseline: 1.4818x; 1.0637x over previous
"""Trainium2 Bass kernel for EpipolarAttention (B=2, C=3, H=W=64, N=4096).

Factorization (validated against the reference to ~5e-6 absmax-rel):
  d_epipolar[i,j]^2 = ||a_i||^2 - (a_i . u_j)^2        (cross-product identity)
                    = alpha_i . beta_j                  (rank-6 bilinear form)
  with alpha = [ax^2,ay^2,az^2,2axay,2axaz,2ayaz],
       beta  = [1-ux^2,1-uy^2,1-uz^2,-uxuy,-uxuz,-uyuz],
       a_i = f_src[:,i]-o_proj, u_j = diff_j/||diff_j||.

  Row softmax (axis j): E1 = exp(50*d - 50*||a_i||)     (bound shift, no max pass)
  W = 1 - E1/rowsum(E1); Aw = A*W with A = f_src^T f_tar (rank-3 matmul)
  Col softmax (axis i): P = exp(Aw - 20) (static shift), colsum all-reduced over
  the 8 cores (each owns 512 rows of i), normalization folded into the final
  matmul rhs: attended^T[c,i] = sum_j (f_src[c,j]/colsum[j]) * P^T[j,i].

Sharding: each core processes a 512-row i-stripe of both batches. The final
contraction runs over j which must sit on PE partitions, so W (values in
[0,1]) is transposed via the 2-byte XBAR DMA transpose in bf16 and Aw^T is
rebuilt as A^T (rank-3 matmul, exact) * W^T. The only cross-core traffic is a
16KB AllReduce of colsum per batch.
"""
import numpy as np

B, C, H, Wd = 2, 3, 64, 64
N = H * Wd                     # 4096
NCORES = 8
ST = N // NCORES               # 512 rows per core
NIB = ST // 128                # 4 partition blocks per stripe
JT = N // 128                  # 32 column chunks of 128 (j on partitions)
NG = 4                         # P^T groups (8 column chunks each)
KG = JT // NG                  # 8 column chunks per group
SH = 50.0                      # sharpness
MSHIFT = 20.0                  # static shift for the column softmax
EPS_D2 = 1e-3                  # clamps fp-negative d^2 before sqrt

# per-batch constant layout "cst" [B, 128, CWB] (f32 words per partition)
OFF_GSRC = N + ST              # 4608: gsrc [128, 96]
OFF_BIAS = OFF_GSRC + JT * 3   # 4704: bias1 [128, 4]
OFF_EPS = OFF_BIAS + NIB       # 4708: [eps, -MSHIFT]
CWB = OFF_EPS + 2              # 4710

_cache = {}


def _build(reps=1, skip_collective=False):
    import concourse.bass as bass
    import concourse.bacc as bacc
    import concourse.mybir as mybir
    import concourse.tile as tile

    f32 = mybir.dt.float32
    bf16 = mybir.dt.bfloat16
    AF = mybir.ActivationFunctionType
    OP = mybir.AluOpType

    # Bacc (not raw Bass): its compile() splits multi-waits into event
    # semaphores — HW instructions carry at most one sync wait.
    nc = bacc.Bacc("TRN2", target_bir_lowering=False, num_devices=NCORES)

    d_cst = nc.dram_tensor("cst", [B, 128, CWB], f32, kind="ExternalInput")
    d_out = nc.dram_tensor("outp", [B, 3, ST], f32, kind="ExternalOutput")

    with tile.TileContext(nc) as tc:
        with (
            tc.tile_pool(name="consts", bufs=2) as consts,
            tc.tile_pool(name="rowd", bufs=2) as rowd,      # d -> E1 -> W (in place)
            tc.tile_pool(name="half", bufs=2) as half,      # W cast to bf16
            tc.tile_pool(name="halfT", bufs=4) as halfT,    # W^T groups (bf16)
            tc.tile_pool(name="rowP", bufs=4) as rowP,      # Aw^T -> P^T groups
            tc.tile_pool(name="small", bufs=2) as small,
            tc.tile_pool(name="psmm", bufs=2, space="PSUM") as psmm,
            tc.tile_pool(name="dram", bufs=2, space="DRAM") as drampool,
        ):
            for _rep in range(reps):
              for b in range(B):
                # ---- per-batch constants (single DMA). PE needs lhsT/rhs on
                # the same base partition in {0,32,64,96}: alpha+beta at base
                # 0, fsrc+ftar at base 32, at distinct free offsets.
                cstb = consts.tile([128, CWB], f32, tag="cstb")
                nc.sync.dma_start(out=cstb[:], in_=d_cst.ap()[b])
                beta_sb = cstb[0:6, 0:N]
                alpha_sb = cstb[0:6, N:N + ST]
                ftar_sb = cstb[32:35, 0:N]
                fsrc_sb = cstb[32:35, N:N + ST]
                gsrc_sb = cstb[:, OFF_GSRC:OFF_GSRC + JT * 3]
                bias1_sb = cstb[:, OFF_BIAS:OFF_BIAS + NIB]
                epsb_sb = cstb[:, OFF_EPS:OFF_EPS + 1]
                mshift_sb = cstb[:, OFF_EPS + 1:OFF_EPS + 2]

                # ---- phase B+C1 per i-block: d = sqrt(alpha.beta + eps);
                # E1 = exp(50d - 50|a|) in place (rs fused via accum_out);
                # W = 1 - E1/rs in place; cast bf16; XBAR-transpose ----------
                rs_t = small.tile([128, NIB], f32, tag="rs")
                ir_t = small.tile([128, NIB], f32, tag="ir")
                hiT = [
                    halfT.tile([128, KG, NIB, 128], bf16, tag="hiT",
                               name=f"hiT_{b}_{g}")
                    for g in range(NG)
                ]
                for ib in range(NIB):
                    dE = rowd.tile([128, N], f32, tag="dE")
                    for hf in range(2):
                        ps = psmm.tile([128, 2048], f32, tag="mm")
                        for h in range(4):
                            jb = hf * 4 + h
                            nc.tensor.matmul(
                                ps[:, h * 512:(h + 1) * 512],
                                lhsT=alpha_sb[:, ib * 128:(ib + 1) * 128],
                                rhs=beta_sb[:, jb * 512:(jb + 1) * 512],
                                start=True, stop=True,
                            )
                        nc.scalar.activation(
                            dE[:, hf * 2048:(hf + 1) * 2048], ps[:],
                            AF.Sqrt, bias=epsb_sb, scale=1.0,
                        )
                    nc.scalar.activation(
                        dE[:], dE[:],
                        AF.Exp, bias=bias1_sb[:, ib:ib + 1], scale=SH,
                        accum_out=rs_t[:, ib:ib + 1],
                    )
                    nc.vector.reciprocal(
                        ir_t[:, ib:ib + 1], rs_t[:, ib:ib + 1])
                    # W_neg = E1/rs - 1 = -W, cast to bf16 in the same pass;
                    # the P^T exp later applies scale=-1 to undo the sign.
                    hi = half.tile([128, N], bf16, tag="half")
                    nc.vector.tensor_scalar(
                        hi[:], dE[:],
                        scalar1=ir_t[:, ib:ib + 1], scalar2=1.0,
                        op0=OP.mult, op1=OP.subtract,
                    )
                    for g in range(NG):
                        nc.sync.dma_start_transpose(
                            hiT[g][:, :, ib, :],
                            hi[:, g * 1024:(g + 1) * 1024],
                        )

                # ---- phase C2 per group: Aw^T = A^T * W^T; P^T = exp(Aw^T-20)
                # in place; colsum partials by free-axis reduce ---------------
                colsum_t = small.tile([128, JT], f32, tag="colsum")
                pt_tiles = []
                for g in range(NG):
                    awt = rowP.tile([128, N], f32, tag="rowP", name=f"awt_{b}_{g}")
                    for hf in range(2):
                        ps = psmm.tile([128, 2048], f32, tag="mm")
                        for h in range(4):
                            jt = g * KG + hf * 4 + h
                            nc.tensor.matmul(
                                ps[:, h * 512:(h + 1) * 512],
                                lhsT=ftar_sb[:, jt * 128:(jt + 1) * 128],
                                rhs=fsrc_sb[:],
                                start=True, stop=True,
                            )
                        nc.vector.tensor_tensor(
                            awt[:, hf * 2048:(hf + 1) * 2048], ps[:],
                            hiT[g][:, hf * 4:hf * 4 + 4, :, :].rearrange(
                                "p k i2 q2 -> p (k i2 q2)"),
                            op=OP.mult,
                        )
                    # awt holds A^T * (-W^T); exp(-x - 20) restores the sign
                    nc.scalar.activation(
                        awt[:], awt[:], AF.Exp, bias=mshift_sb, scale=-1.0,
                    )
                    nc.vector.tensor_reduce(
                        colsum_t[:, g * KG:(g + 1) * KG],
                        awt[:].rearrange("p (k i) -> p k i", k=KG),
                        axis=mybir.AxisListType.X, op=OP.add,
                    )
                    pt_tiles.append(awt)

                # ---- phase R: AllReduce colsum over the 8 cores -------------
                cc_in = drampool.tile([128, JT], f32, tag="ccin")
                cc_out = drampool.tile([128, JT], f32, tag="ccout")
                nc.sync.dma_start(out=cc_in[:], in_=colsum_t[:])
                if skip_collective:
                    nc.sync.dma_start(out=cc_out[:], in_=cc_in[:])
                else:
                    nc.gpsimd.collective_compute(
                        "AllReduce", OP.add,
                        replica_groups=[list(range(NCORES))],
                        ins=[cc_in.opt()], outs=[cc_out.opt()],
                    )
                cstot_t = small.tile([128, JT], f32, tag="cstot")
                nc.sync.dma_start(out=cstot_t[:], in_=cc_out[:])

                icol_t = small.tile([128, JT], f32, tag="icol")
                nc.vector.reciprocal(icol_t[:], cstot_t[:])
                # g[p, (jt,c)] = f_srcT[p, (jt,c)] / colsum[p, jt]
                g_t = small.tile([128, JT, 3], f32, tag="gt")
                icol_rep = bass.AP(
                    tensor=icol_t[:].tensor,
                    offset=icol_t[:].offset,
                    ap=[icol_t[:].ap[0], icol_t[:].ap[1], [0, 3]],
                )
                nc.vector.tensor_tensor(
                    g_t[:],
                    gsrc_sb.rearrange("p (t c) -> p t c", c=3),
                    icol_rep, op=OP.mult,
                )

                # ---- phase F: attended^T[c,i] = sum_j g[j,c] P^T[j,i] -------
                ps_att = psmm.tile([128, 2048], f32, tag="mm", name=f"att_{b}")
                for jt in range(JT):
                    g_, k_ = divmod(jt, KG)
                    nc.tensor.matmul(
                        ps_att[0:3, 0:ST],
                        lhsT=g_t[:, jt, :],
                        rhs=pt_tiles[g_][:, k_ * 512:(k_ + 1) * 512],
                        start=(jt == 0), stop=(jt == JT - 1),
                    )
                out_sb = small.tile([3, ST], f32, tag="outsb")
                nc.vector.tensor_copy(out_sb[:], ps_att[0:3, 0:ST])
                nc.sync.dma_start(out=d_out.ap()[b], in_=out_sb[:])

    nc.compile()
    return nc


def _host_prep(f_tar, f_src, K, R, t):
    """Geometry on host (CPU jax mirrors the reference's f32 ops), then the
    per-core device input arrays."""
    import jax
    import jax.numpy as jnp

    cpu = jax.devices("cpu")[0]

    def geom(K, R, t):
        ix, iy = jnp.meshgrid(jnp.arange(H), jnp.arange(Wd), indexing="ij")
        pix = jnp.stack(
            [ix.reshape(-1), iy.reshape(-1), jnp.ones(N, dtype=jnp.int32)], axis=0
        ).astype(jnp.float32)
        K_inv = jnp.linalg.inv(K)
        cam_rays = jnp.einsum("bij,jn->bin", K_inv, pix)
        tar_proj = jnp.einsum(
            "bij,bjn->bin", K, jnp.einsum("bij,bjn->bin", R, cam_rays) + t
        )
        o_proj = jnp.einsum("bij,bjn->bin", K, t)
        return tar_proj - o_proj, o_proj

    diff, o_proj = jax.jit(geom, backend="cpu")(
        jax.device_put(np.asarray(K), cpu),
        jax.device_put(np.asarray(R), cpu),
        jax.device_put(np.asarray(t), cpu),
    )
    diff = np.asarray(diff)
    o_proj = np.asarray(o_proj)

    f_src_flat = np.asarray(f_src).reshape(B, C, N)
    f_tar_flat = np.asarray(f_tar).reshape(B, C, N)

    diff_norm = np.sqrt((diff ** 2).sum(axis=1, keepdims=True)).astype(np.float32)
    u = (diff / diff_norm).astype(np.float32)
    a = (f_src_flat - o_proj).astype(np.float32)
    na = (a ** 2).sum(axis=1).astype(np.float32)          # (B,N)
    sqna = np.sqrt(na).astype(np.float32)

    alpha = np.stack(
        [a[:, 0] ** 2, a[:, 1] ** 2, a[:, 2] ** 2,
         2 * a[:, 0] * a[:, 1], 2 * a[:, 0] * a[:, 2], 2 * a[:, 1] * a[:, 2]],
        axis=1,
    ).astype(np.float32)                                   # (B,6,N)
    beta = np.stack(
        [1 - u[:, 0] ** 2, 1 - u[:, 1] ** 2, 1 - u[:, 2] ** 2,
         -u[:, 0] * u[:, 1], -u[:, 0] * u[:, 2], -u[:, 1] * u[:, 2]],
        axis=1,
    ).astype(np.float32)                                   # (B,6,N)

    bias1 = (-SH * sqna).astype(np.float32)                # (B,N)
    # gsrc[b, p, jt*3+c] = f_src[b, c, jt*128+p]
    gsrc = np.ascontiguousarray(
        f_src_flat.transpose(0, 2, 1).reshape(B, JT, 128, 3).transpose(0, 2, 1, 3)
    ).reshape(B, 128, JT * 3).astype(np.float32)

    in_maps = []
    for r in range(NCORES):
        sl = slice(r * ST, (r + 1) * ST)
        cst = np.zeros((B, 128, CWB), dtype=np.float32)
        cst[:, 0:6, 0:N] = beta.astype(np.float32)
        cst[:, 0:6, N:N + ST] = alpha[:, :, sl]
        cst[:, 32:35, 0:N] = f_tar_flat
        cst[:, 32:35, N:N + ST] = f_src_flat[:, :, sl]
        cst[:, :, OFF_GSRC:OFF_GSRC + JT * 3] = gsrc
        cst[:, :, OFF_BIAS:OFF_BIAS + NIB] = (
            bias1[:, sl].reshape(B, NIB, 128).transpose(0, 2, 1))
        cst[:, :, OFF_EPS] = EPS_D2
        cst[:, :, OFF_EPS + 1] = -MSHIFT
        in_maps.append({"cst": cst})
    return in_maps


def _run(in_maps, reps=1, skip_collective=False):
    from concourse.bass_utils import run_bass_kernel_spmd

    key = ("nc", reps, skip_collective)
    if key not in _cache:
        _cache[key] = _build(reps, skip_collective)
    res = run_bass_kernel_spmd(
        _cache[key], in_maps, list(range(NCORES))
    )
    return res


def kernel(f_tar, f_src, K, R, t, _reps=1):
    in_maps = _host_prep(f_tar, f_src, K, R, t)
    res = _run(in_maps, reps=_reps)
    att_T = np.empty((B, 3, N), dtype=np.float32)
    for r in range(NCORES):
        att_T[:, :, r * ST:(r + 1) * ST] = res.results[r]["outp"]
    out = att_T.transpose(0, 2, 1).reshape(B, C, H, Wd)
    kernel._last_results = res
    return out


# revision 28
# speedup vs baseline: 2.0363x; 1.3743x over previous
"""Trainium2 Bass kernel for EpipolarAttention (B=2, C=3, H=W=64, N=4096).

Factorization (validated against the reference to ~5e-6 absmax-rel):
  d_epipolar[i,j]^2 = ||a_i||^2 - (a_i . u_j)^2        (cross-product identity)
                    = alpha_i . beta_j                  (rank-6 bilinear form)
  with alpha = [ax^2,ay^2,az^2,2axay,2axaz,2ayaz],
       beta  = [1-ux^2,1-uy^2,1-uz^2,-uxuy,-uxuz,-uyuz],
       a_i = f_src[:,i]-o_proj, u_j = diff_j/||diff_j||.

  Row softmax (axis j): E1 = exp(50*d - 50*||a_i||)     (bound shift, no max pass)
  W = 1 - E1/rowsum(E1); Aw = A*W with A = f_src^T f_tar (rank-3 matmul)
  Col softmax (axis i): P = exp(Aw - 20) (static shift), colsum all-reduced over
  the 8 cores (each owns 512 rows of i), normalization folded into the final
  matmul rhs: attended^T[c,i] = sum_j (f_src[c,j]/colsum[j]) * P^T[j,i].

Sharding: each core processes a 512-row i-stripe of both batches. The final
contraction runs over j which must sit on PE partitions, so W (values in
[0,1]) is transposed via the 2-byte XBAR DMA transpose in bf16 and Aw^T is
rebuilt as A^T (rank-3 matmul, exact) * W^T. The only cross-core traffic is a
16KB AllReduce of colsum per batch.
"""
import numpy as np

B, C, H, Wd = 2, 3, 64, 64
N = H * Wd                     # 4096
NCORES = 8
ST = N // NCORES               # 512 rows per core
NIB = ST // 128                # 4 partition blocks per stripe
JT = N // 128                  # 32 column chunks of 128 (j on partitions)
NG = 4                         # P^T groups (8 column chunks each)
KG = JT // NG                  # 8 column chunks per group
SH = 50.0                      # sharpness
MSHIFT = 20.0                  # static shift for the column softmax
EPS_D2 = 1e-3                  # clamps fp-negative d^2 before sqrt

# per-batch constant layout "cst" [B, 128, CWB] (f32 words per partition)
OFF_GSRC = N + ST              # 4608: gsrc [128, 96]
OFF_BIAS = OFF_GSRC + JT * 3   # 4704: bias1 [128, 4]
OFF_EPS = OFF_BIAS + NIB       # 4708: [eps, -MSHIFT]
CWB = OFF_EPS + 2              # 4710

_cache = {}


def _build(reps=1, skip_collective=False):
    import concourse.bass as bass
    import concourse.bacc as bacc
    import concourse.mybir as mybir
    import concourse.tile as tile

    f32 = mybir.dt.float32
    bf16 = mybir.dt.bfloat16
    AF = mybir.ActivationFunctionType
    OP = mybir.AluOpType

    # Bacc (not raw Bass): its compile() splits multi-waits into event
    # semaphores — HW instructions carry at most one sync wait.
    nc = bacc.Bacc("TRN2", target_bir_lowering=False, num_devices=NCORES)

    d_cst = nc.dram_tensor("cst", [B, 128, CWB], f32, kind="ExternalInput")
    d_out = nc.dram_tensor("outp", [B, 3, ST], f32, kind="ExternalOutput")

    with tile.TileContext(nc) as tc:
        with (
            tc.tile_pool(name="consts", bufs=2) as consts,
            tc.tile_pool(name="rowd", bufs=2) as rowd,      # d -> E1 -> W (in place)
            tc.tile_pool(name="half", bufs=2) as half,      # W cast to bf16
            tc.tile_pool(name="halfT", bufs=4) as halfT,    # W^T groups (bf16)
            tc.tile_pool(name="rowP", bufs=4) as rowP,      # Aw^T -> P^T groups
            tc.tile_pool(name="small", bufs=2) as small,
            tc.tile_pool(name="psmm", bufs=2, space="PSUM") as psmm,
            tc.tile_pool(name="dram", bufs=2, space="DRAM") as drampool,
        ):
            for _rep in range(reps):
              for b in range(B):
                # ---- per-batch constants (single DMA). PE needs lhsT/rhs on
                # the same base partition in {0,32,64,96}: alpha+beta at base
                # 0, fsrc+ftar at base 32, at distinct free offsets.
                cstb = consts.tile([128, CWB], f32, tag="cstb")
                nc.sync.dma_start(out=cstb[:], in_=d_cst.ap()[b])
                beta_sb = cstb[0:6, 0:N]
                alpha_sb = cstb[0:6, N:N + ST]
                ftar_sb = cstb[32:35, 0:N]
                fsrc_sb = cstb[32:35, N:N + ST]
                gsrc_sb = cstb[:, OFF_GSRC:OFF_GSRC + JT * 3]
                bias1_sb = cstb[:, OFF_BIAS:OFF_BIAS + NIB]
                epsb_sb = cstb[:, OFF_EPS:OFF_EPS + 1]
                mshift_sb = cstb[:, OFF_EPS + 1:OFF_EPS + 2]

                # ---- phase B+C1 per i-block: d = sqrt(alpha.beta + eps);
                # E1 = exp(50d - 50|a|) in place (rs fused via accum_out);
                # W = 1 - E1/rs in place; cast bf16; XBAR-transpose ----------
                rs_t = small.tile([128, NIB], f32, tag="rs")
                ir_t = small.tile([128, NIB], f32, tag="ir")
                hiT = [
                    halfT.tile([128, KG, NIB, 128], bf16, tag="hiT",
                               name=f"hiT_{b}_{g}")
                    for g in range(NG)
                ]
                for ib in range(NIB):
                    dE = rowd.tile([128, N], f32, tag="dE")
                    for hf in range(2):
                        ps = psmm.tile([128, 2048], f32, tag="mm")
                        for h in range(4):
                            jb = hf * 4 + h
                            nc.tensor.matmul(
                                ps[:, h * 512:(h + 1) * 512],
                                lhsT=alpha_sb[:, ib * 128:(ib + 1) * 128],
                                rhs=beta_sb[:, jb * 512:(jb + 1) * 512],
                                start=True, stop=True,
                            )
                        nc.scalar.activation(
                            dE[:, hf * 2048:(hf + 1) * 2048], ps[:],
                            AF.Sqrt, bias=epsb_sb, scale=1.0,
                        )
                    nc.scalar.activation(
                        dE[:], dE[:],
                        AF.Exp, bias=bias1_sb[:, ib:ib + 1], scale=SH,
                        accum_out=rs_t[:, ib:ib + 1],
                    )
                    nc.vector.reciprocal(
                        ir_t[:, ib:ib + 1], rs_t[:, ib:ib + 1])
                    # W_neg = E1/rs - 1 = -W, cast to bf16 in the same pass;
                    # the P^T exp later applies scale=-1 to undo the sign.
                    hi = half.tile([128, N], bf16, tag="half")
                    nc.vector.tensor_scalar(
                        hi[:], dE[:],
                        scalar1=ir_t[:, ib:ib + 1], scalar2=1.0,
                        op0=OP.mult, op1=OP.subtract,
                    )
                    for g in range(NG):
                        nc.sync.dma_start_transpose(
                            hiT[g][:, :, ib, :],
                            hi[:, g * 1024:(g + 1) * 1024],
                        )

                # ---- phase C2 per group: Aw^T = A^T * W^T; P^T = exp(Aw^T-20)
                # in place; colsum partials by free-axis reduce ---------------
                colsum_t = small.tile([128, JT], f32, tag="colsum")
                pt_tiles = []
                for g in range(NG):
                    awt = rowP.tile([128, N], f32, tag="rowP", name=f"awt_{b}_{g}")
                    for hf in range(2):
                        ps = psmm.tile([128, 2048], f32, tag="mm")
                        for h in range(4):
                            jt = g * KG + hf * 4 + h
                            nc.tensor.matmul(
                                ps[:, h * 512:(h + 1) * 512],
                                lhsT=ftar_sb[:, jt * 128:(jt + 1) * 128],
                                rhs=fsrc_sb[:],
                                start=True, stop=True,
                            )
                        nc.vector.tensor_tensor(
                            awt[:, hf * 2048:(hf + 1) * 2048], ps[:],
                            hiT[g][:, hf * 4:hf * 4 + 4, :, :].rearrange(
                                "p k i2 q2 -> p (k i2 q2)"),
                            op=OP.mult,
                        )
                    # awt holds A^T * (-W^T); exp(-x - 20) restores the sign
                    nc.scalar.activation(
                        awt[:], awt[:], AF.Exp, bias=mshift_sb, scale=-1.0,
                    )
                    nc.vector.tensor_reduce(
                        colsum_t[:, g * KG:(g + 1) * KG],
                        awt[:].rearrange("p (k i) -> p k i", k=KG),
                        axis=mybir.AxisListType.X, op=OP.add,
                    )
                    pt_tiles.append(awt)

                # ---- phase R: AllReduce colsum over the 8 cores -------------
                cc_in = drampool.tile([128, JT], f32, tag="ccin")
                cc_out = drampool.tile([128, JT], f32, tag="ccout")
                nc.sync.dma_start(out=cc_in[:], in_=colsum_t[:])
                if skip_collective:
                    nc.sync.dma_start(out=cc_out[:], in_=cc_in[:])
                else:
                    nc.gpsimd.collective_compute(
                        "AllReduce", OP.add,
                        replica_groups=[list(range(NCORES))],
                        ins=[cc_in.opt()], outs=[cc_out.opt()],
                    )
                cstot_t = small.tile([128, JT], f32, tag="cstot")
                nc.sync.dma_start(out=cstot_t[:], in_=cc_out[:])

                icol_t = small.tile([128, JT], f32, tag="icol")
                nc.vector.reciprocal(icol_t[:], cstot_t[:])
                # g[p, (jt,c)] = f_srcT[p, (jt,c)] / colsum[p, jt]
                g_t = small.tile([128, JT, 3], f32, tag="gt")
                icol_rep = bass.AP(
                    tensor=icol_t[:].tensor,
                    offset=icol_t[:].offset,
                    ap=[icol_t[:].ap[0], icol_t[:].ap[1], [0, 3]],
                )
                nc.vector.tensor_tensor(
                    g_t[:],
                    gsrc_sb.rearrange("p (t c) -> p t c", c=3),
                    icol_rep, op=OP.mult,
                )

                # ---- phase F: attended^T[c,i] = sum_j g[j,c] P^T[j,i] -------
                ps_att = psmm.tile([128, 2048], f32, tag="mm", name=f"att_{b}")
                for jt in range(JT):
                    g_, k_ = divmod(jt, KG)
                    nc.tensor.matmul(
                        ps_att[0:3, 0:ST],
                        lhsT=g_t[:, jt, :],
                        rhs=pt_tiles[g_][:, k_ * 512:(k_ + 1) * 512],
                        start=(jt == 0), stop=(jt == JT - 1),
                    )
                out_sb = small.tile([3, ST], f32, tag="outsb")
                nc.vector.tensor_copy(out_sb[:], ps_att[0:3, 0:ST])
                nc.sync.dma_start(out=d_out.ap()[b], in_=out_sb[:])

    nc.compile()
    return nc


def _host_prep(f_tar, f_src, K, R, t):
    """Geometry on host (CPU jax mirrors the reference's f32 ops), then the
    per-core device input arrays."""
    import jax
    import jax.numpy as jnp

    cpu = jax.devices("cpu")[0]

    def geom(K, R, t):
        ix, iy = jnp.meshgrid(jnp.arange(H), jnp.arange(Wd), indexing="ij")
        pix = jnp.stack(
            [ix.reshape(-1), iy.reshape(-1), jnp.ones(N, dtype=jnp.int32)], axis=0
        ).astype(jnp.float32)
        K_inv = jnp.linalg.inv(K)
        cam_rays = jnp.einsum("bij,jn->bin", K_inv, pix)
        tar_proj = jnp.einsum(
            "bij,bjn->bin", K, jnp.einsum("bij,bjn->bin", R, cam_rays) + t
        )
        o_proj = jnp.einsum("bij,bjn->bin", K, t)
        return tar_proj - o_proj, o_proj

    diff, o_proj = jax.jit(geom, backend="cpu")(
        jax.device_put(np.asarray(K), cpu),
        jax.device_put(np.asarray(R), cpu),
        jax.device_put(np.asarray(t), cpu),
    )
    diff = np.asarray(diff)
    o_proj = np.asarray(o_proj)

    f_src_flat = np.asarray(f_src).reshape(B, C, N)
    f_tar_flat = np.asarray(f_tar).reshape(B, C, N)

    diff_norm = np.sqrt((diff ** 2).sum(axis=1, keepdims=True)).astype(np.float32)
    u = (diff / diff_norm).astype(np.float32)
    a = (f_src_flat - o_proj).astype(np.float32)
    na = (a ** 2).sum(axis=1).astype(np.float32)          # (B,N)
    sqna = np.sqrt(na).astype(np.float32)

    alpha = np.stack(
        [a[:, 0] ** 2, a[:, 1] ** 2, a[:, 2] ** 2,
         2 * a[:, 0] * a[:, 1], 2 * a[:, 0] * a[:, 2], 2 * a[:, 1] * a[:, 2]],
        axis=1,
    ).astype(np.float32)                                   # (B,6,N)
    beta = np.stack(
        [1 - u[:, 0] ** 2, 1 - u[:, 1] ** 2, 1 - u[:, 2] ** 2,
         -u[:, 0] * u[:, 1], -u[:, 0] * u[:, 2], -u[:, 1] * u[:, 2]],
        axis=1,
    ).astype(np.float32)                                   # (B,6,N)

    bias1 = (-SH * sqna).astype(np.float32)                # (B,N)
    # gsrc[b, p, jt*3+c] = f_src[b, c, jt*128+p]
    gsrc = np.ascontiguousarray(
        f_src_flat.transpose(0, 2, 1).reshape(B, JT, 128, 3).transpose(0, 2, 1, 3)
    ).reshape(B, 128, JT * 3).astype(np.float32)

    in_maps = []
    for r in range(NCORES):
        sl = slice(r * ST, (r + 1) * ST)
        cst = np.zeros((B, 128, CWB), dtype=np.float32)
        cst[:, 0:6, 0:N] = beta.astype(np.float32)
        cst[:, 0:6, N:N + ST] = alpha[:, :, sl]
        cst[:, 32:35, 0:N] = f_tar_flat
        cst[:, 32:35, N:N + ST] = f_src_flat[:, :, sl]
        cst[:, :, OFF_GSRC:OFF_GSRC + JT * 3] = gsrc
        cst[:, :, OFF_BIAS:OFF_BIAS + NIB] = (
            bias1[:, sl].reshape(B, NIB, 128).transpose(0, 2, 1))
        cst[:, :, OFF_EPS] = EPS_D2
        cst[:, :, OFF_EPS + 1] = -MSHIFT
        in_maps.append({"cst": cst})
    return in_maps


def _run(in_maps, reps=1, skip_collective=False):
    import time
    from concourse.bass_utils import run_bass_kernel_spmd

    key = ("nc", reps, skip_collective)
    if key not in _cache:
        _cache[key] = _build(reps, skip_collective)
    last = None
    for attempt in range(3):
        try:
            return run_bass_kernel_spmd(
                _cache[key], in_maps, list(range(NCORES))
            )
        except Exception as e:  # transient relay/device hiccups
            last = e
            time.sleep(15 * (attempt + 1))
    raise last


def kernel(f_tar, f_src, K, R, t, _reps=1):
    in_maps = _host_prep(f_tar, f_src, K, R, t)
    res = _run(in_maps, reps=_reps)
    att_T = np.empty((B, 3, N), dtype=np.float32)
    for r in range(NCORES):
        att_T[:, :, r * ST:(r + 1) * ST] = res.results[r]["outp"]
    out = att_T.transpose(0, 2, 1).reshape(B, C, H, Wd)
    kernel._last_results = res
    return out
